# revision 4
# baseline (speedup 1.0000x reference)
"""GATv2 x5 on 8 TRN2 cores — v2.

Per-core: nodes LPT-balanced into 49 tiles of 128 (K edge chunks per tile,
self-loops as a free 'identity' chunk). Per layer: prologue computes
xl_a = h @ (Wl diag(a) perm) and xr_a rows (bf16), AllGathers xl rows
[129 cols: xl_a|Ptilde]; edge stage per tile: indirect-gathers xl rows per
chunk, computes scores via PE adds (XL + mask@xr) into PSUM, Act-copies to
bf16, sigma-split abs-reduces give e = 0.6p + 0.4*sum(a*|u|) (LeakyReLU
algebra: LReLU(u) = 0.6u + 0.4|u|; exp(0.6 q_dst) cancels in softmax),
then one-hot-scaled scatter matmuls accumulate numerator+denominator.
"""
import sys
import numpy as np

sys.path.insert(0, "/opt/trn_rl_repo")

import ml_dtypes
import concourse.bass as bass
import concourse.bacc as bacc
import concourse.mybir as mybir
import concourse.tile as tile
from concourse.bass_utils import run_bass_kernel_spmd

F32 = mybir.dt.float32
BF16 = mybir.dt.bfloat16
I32 = mybir.dt.int32
AF = mybir.ActivationFunctionType
OP = mybir.AluOpType
AX = mybir.AxisListType

N = 50000
DIN = 7
D = 128
T = 5
CORES = 8
TILES = 49
SHP = TILES * 128          # 6272 padded nodes per core
NPG = CORES * SHP          # 50176 global padded rows
NEG = 0.2                  # LReLU(u) = 0.6u + 0.4|u| for slope 0.2
GRP = 4                    # score chunks per PSUM bank group


def _build_nc(K: int, ms: tuple, repeat: int = 1, ablate: tuple = ()):
    CH = K + 1  # + self chunk
    nc = bacc.Bacc("TRN2", target_bir_lowering=False, debug=False,
                   num_devices=CORES, num_swdge_queues=4)

    xT_own = nc.dram_tensor("xT_own", [DIN, SHP], BF16, kind="ExternalInput")
    w_all = nc.dram_tensor("w_all", [T, 128, 2 * D + 1], BF16,
                           kind="ExternalInput")
    corr_d = nc.dram_tensor("corr_d", [T, D], F32, kind="ExternalInput")
    bout_d = nc.dram_tensor("bout_d", [D, T], F32, kind="ExternalInput")
    gsrc_d = nc.dram_tensor("gsrc_d", [128, TILES * K], I32,
                            kind="ExternalInput")
    dstl_d = nc.dram_tensor("dstl_d", [128, TILES * CH], F32,
                            kind="ExternalInput")
    mask_d = nc.dram_tensor("mask_d", [128, TILES * CH * 128], BF16,
                            kind="ExternalInput")

    out_t = nc.dram_tensor("out", [SHP, D], F32, kind="ExternalOutput")

    with tile.TileContext(nc) as tc:
        with (
            tc.tile_pool(name="pers", bufs=1) as pers,
            tc.tile_pool(name="wl", bufs=2) as wl,
            tc.tile_pool(name="xe", bufs=3) as xe,       # XL / U / mask tiles
            tc.tile_pool(name="sb", bufs=4) as sbp,      # small tiles
            tc.tile_pool(name="pro", bufs=2, space="PSUM") as ppro,
            tc.tile_pool(name="pss", bufs=2, space="PSUM") as psc,
            tc.tile_pool(name="psa", bufs=2, space="PSUM") as psa,
            tc.tile_pool(name="pst", bufs=2, space="PSUM") as pstp,
            tc.tile_pool(name="dram", bufs=2, space="DRAM") as dp,
        ):
            # ---- persistent setup ----
            iota_c = pers.tile([128, 1], I32)
            nc.gpsimd.iota(iota_c[:], pattern=[[0, 1]], base=0,
                           channel_multiplier=1)
            iota_cf = pers.tile([128, 1], F32)
            nc.vector.tensor_copy(iota_cf[:], iota_c[:])
            iota_r = pers.tile([128, 128], I32)
            nc.gpsimd.iota(iota_r[:], pattern=[[1, 128]], base=0,
                           channel_multiplier=0)
            iota_rb = pers.tile([128, 128], BF16)
            nc.vector.tensor_copy(iota_rb[:], iota_r[:])
            ident_b = pers.tile([128, 128], BF16)
            nc.vector.tensor_scalar(
                out=ident_b[:], in0=iota_rb[:], scalar1=iota_cf[:],
                scalar2=None, op0=OP.is_equal)

            gsrc_sb = pers.tile([128, TILES * K], I32)
            nc.sync.dma_start(out=gsrc_sb[:], in_=gsrc_d[:, :])
            dstl_sb = pers.tile([128, TILES * CH], F32)
            nc.sync.dma_start(out=dstl_sb[:], in_=dstl_d[:, :])
            xT_sb = pers.tile([DIN, SHP], BF16)
            nc.sync.dma_start(out=xT_sb[:], in_=xT_own[:, :])

            hT = pers.tile([128, SHP], BF16)
            xl_store = pers.tile([128, TILES, D + 1], BF16)
            xr_store = pers.tile([128, TILES, D], BF16)

            for rep in range(repeat):
              for l in range(T):
                m = ms[l]
                # ---- per-layer constants ----
                w_sb = wl.tile([128, 2 * D + 1], BF16, tag="w")
                nc.sync.dma_start(out=w_sb[:], in_=w_all[l, :, :])
                corr_b = wl.tile([128, 128], F32, tag="corr")
                nc.sync.dma_start(
                    out=corr_b[:],
                    in_=corr_d[l : l + 1, :].partition_broadcast(128))
                bo_col = wl.tile([128, 1], F32, tag="boc")
                nc.sync.dma_start(out=bo_col[:], in_=bout_d[:, l : l + 1])
                if l == T - 1:
                    bo_b = wl.tile([128, 128], F32, tag="bob")
                    nc.sync.dma_start(
                        out=bo_b[:],
                        in_=bout_d[:, l : l + 1].transpose([1, 0])
                        .partition_broadcast(128))

                # ---- prologue: per node tile ----
                xl_cc = dp.tile([SHP, D + 1], BF16, tag="xlcc")
                for t in range(TILES):
                    ps_p = ppro.tile([128, 2 * D + 1], F32, space="PSUM",
                                     tag="pro")
                    if l == 0:
                        nc.tensor.matmul(
                            out=ps_p[:], lhsT=xT_sb[:, t * 128 : (t + 1) * 128],
                            rhs=w_sb[:DIN, :], start=True, stop=True)
                    else:
                        nc.tensor.matmul(
                            out=ps_p[:], lhsT=hT[:, t * 128 : (t + 1) * 128],
                            rhs=w_sb[:], start=True, stop=True)
                    nc.vector.tensor_copy(
                        xl_store[:, t, 0:D], ps_p[:, 0:D])
                    nc.scalar.activation(
                        out=xl_store[:, t, D : D + 1],
                        in_=ps_p[:, D : D + 1], func=AF.Exp, scale=0.6)
                    nc.vector.tensor_copy(
                        xr_store[:, t, :], ps_p[:, D + 1 : 2 * D + 1])
                    nc.sync.dma_start(
                        out=xl_cc[t * 128 : (t + 1) * 128, :],
                        in_=xl_store[:, t, :])

                xl_full = dp.tile([NPG, D + 1], BF16, tag="xlfull",
                                  addr_space="Shared")
                if "ag" not in ablate:
                    nc.gpsimd.collective_compute(
                        "AllGather",
                        OP.bypass,
                        replica_groups=[list(range(CORES))],
                        ins=[xl_cc[:, :].opt()],
                        outs=[xl_full[:, :].opt()],
                    )

                # ---- edge stage ----
                for t in range(TILES):
                    XL = xe.tile([128, CH, D + 2], BF16, tag="XL")
                    nc.vector.memset(XL[:, :, 0:1], 1.0)
                    msk = xe.tile([128, CH * 128], BF16, tag="msk")
                    nc.sync.dma_start(
                        out=msk[:],
                        in_=mask_d[:, t * CH * 128 : (t + 1) * CH * 128])
                    for k in range(K if "gather" not in ablate else 0):
                        gi_inst = nc.gpsimd.indirect_dma_start(
                            out=XL[:, k, 1 : D + 2], out_offset=None,
                            in_=xl_full[:, :],
                            in_offset=bass.IndirectOffsetOnAxis(
                                ap=gsrc_sb[:, t * K + k : t * K + k + 1],
                                axis=0),
                        )
                        qn = k % 4
                        if qn:
                            try:
                                gi_inst.inst.queue = f"qPoolDynamic{qn}"
                            except AttributeError:
                                gi_inst.queue = f"qPoolDynamic{qn}"
                    nc.vector.tensor_copy(
                        XL[:, K, 1 : D + 2], xl_store[:, t, :])

                    # per-group: scores -> ex -> Oc -> scatter (pipelined)
                    xr_t = xr_store[:, t, :]
                    ps_a = psa.tile([128, D + 1], F32, space="PSUM", tag="a")
                    ngrp = (CH + GRP - 1) // GRP
                    for gi in range(ngrp):
                        g0 = gi * GRP
                        gw = min(GRP, CH - g0)
                        ps_s = psc.tile([128, GRP, D], F32, space="PSUM",
                                        tag="s")
                        for j in range(gw):
                            k = g0 + j
                            nc.tensor.matmul(
                                out=ps_s[:, j, :],
                                lhsT=msk[:, k * 128 : (k + 1) * 128],
                                rhs=xr_t, start=True, stop=False)
                            nc.tensor.matmul(
                                out=ps_s[:, j, :],
                                lhsT=ident_b[:],
                                rhs=XL[:, k, 1 : D + 1], start=False, stop=True)
                        U = sbp.tile([128, GRP, D], BF16, tag="U")
                        nc.scalar.activation(
                            out=U[:, :gw, :],
                            in_=ps_s[:, :gw, :], func=AF.Identity)
                        r1 = sbp.tile([128, GRP], F32, tag="r1")
                        r2n = sbp.tile([128, GRP], F32, tag="r2n")
                        if m > 0:
                            nc.vector.tensor_reduce(
                                out=r1[:, :gw], in_=U[:, :gw, 0:m], axis=AX.X,
                                op=OP.add, apply_absolute_value=True)
                        else:
                            nc.vector.memset(r1[:], 0.0)
                        if m < D:
                            nc.vector.tensor_reduce(
                                out=r2n[:, :gw], in_=U[:, :gw, m:D], axis=AX.X,
                                op=OP.add, apply_absolute_value=True,
                                negate=True)
                        else:
                            nc.vector.memset(r2n[:], 0.0)
                        e4 = sbp.tile([128, GRP], F32, tag="e4")
                        nc.vector.tensor_tensor(
                            out=e4[:, :gw], in0=r1[:, :gw], in1=r2n[:, :gw],
                            op=OP.add)
                        exg = sbp.tile([128, GRP], F32, tag="ex")
                        nc.scalar.activation(
                            out=exg[:, :gw], in_=e4[:, :gw], func=AF.Exp,
                            scale=0.4)
                        nc.vector.tensor_tensor(
                            out=exg[:, :gw], in0=exg[:, :gw],
                            in1=XL[:, g0 : g0 + gw, D + 1], op=OP.mult)
                        for j in range(gw):
                            k = g0 + j
                            Oc = sbp.tile([128, 128], BF16, tag="Oc")
                            nc.vector.tensor_scalar(
                                out=Oc[:], in0=iota_rb[:],
                                scalar1=dstl_sb[:, t * CH + k : t * CH + k + 1],
                                scalar2=exg[:, j : j + 1],
                                op0=OP.is_equal, op1=OP.mult)
                            nc.tensor.matmul(
                                out=ps_a[:], lhsT=Oc[:],
                                rhs=XL[:, k, 0 : D + 1],
                                start=(k == 0), stop=(k == CH - 1))

                    rec = sbp.tile([128, 1], F32, tag="rec")
                    nc.vector.reciprocal(rec[:], ps_a[:, 0:1])
                    if l < T - 1:
                        h1 = sbp.tile([128, D], BF16, tag="h1")
                        nc.vector.scalar_tensor_tensor(
                            out=h1[:], in0=ps_a[:, 1 : D + 1], scalar=rec[:],
                            in1=corr_b[:], op0=OP.mult, op1=OP.mult)
                        ps_t = pstp.tile([128, 128], BF16, space="PSUM",
                                         tag="tr")
                        nc.tensor.transpose(out=ps_t[:], in_=h1[:],
                                            identity=ident_b[:])
                        nc.scalar.activation(
                            out=hT[:, t * 128 : (t + 1) * 128], in_=ps_t[:],
                            func=AF.Relu, bias=bo_col[:], scale=1.0)
                    else:
                        h1f = sbp.tile([128, D], F32, tag="h1f")
                        nc.vector.scalar_tensor_tensor(
                            out=h1f[:], in0=ps_a[:, 1 : D + 1], scalar=rec[:],
                            in1=corr_b[:], op0=OP.mult, op1=OP.mult)
                        o_sb = sbp.tile([128, D], F32, tag="o")
                        nc.vector.tensor_tensor(
                            out=o_sb[:], in0=h1f[:], in1=bo_b[:], op=OP.add)
                        nc.sync.dma_start(
                            out=out_t[t * 128 : (t + 1) * 128, :], in_=o_sb[:])

    nc.compile()
    return nc


def _prep(inputs):
    x = np.asarray(inputs["x"], np.float32)
    ei = np.asarray(inputs["edge_index"]).astype(np.int64)
    Wl0 = np.asarray(inputs["Wl0"], np.float32)
    Wr0 = np.asarray(inputs["Wr0"], np.float32)
    Wl = np.asarray(inputs["Wl"], np.float32)
    Wr = np.asarray(inputs["Wr"], np.float32)
    bl0 = np.asarray(inputs["bl0"], np.float32)
    br0 = np.asarray(inputs["br0"], np.float32)
    bl = np.asarray(inputs["bl"], np.float32)
    br = np.asarray(inputs["br"], np.float32)
    att = np.asarray(inputs["att"], np.float32)
    bias = np.asarray(inputs["bias"], np.float32)

    src, dst = ei[0], ei[1]
    E = src.shape[0]

    # ---- LPT node placement: balance in-degree over 392 (core,tile) bins
    deg = np.bincount(dst, minlength=N).astype(np.int64)
    NB = CORES * TILES
    order = np.argsort(-deg, kind="stable")
    bin_load = np.zeros(NB, np.int64)
    bin_cnt = np.zeros(NB, np.int32)
    bin_nodes = [[] for _ in range(NB)]
    import heapq
    heap = [(0, 0, b) for b in range(NB)]
    heapq.heapify(heap)
    for nd in order:
        while True:
            load, cnt, b = heapq.heappop(heap)
            if (bin_cnt[b] < 128 and load == bin_load[b]
                    and cnt == bin_cnt[b]):
                break
        bin_nodes[b].append(nd)
        bin_cnt[b] += 1
        bin_load[b] += deg[nd]
        if bin_cnt[b] < 128:
            heapq.heappush(heap, (bin_load[b], int(bin_cnt[b]), b))

    # bins -> (core, tile): snake by load for core balance
    border = np.argsort(-bin_load, kind="stable")
    assign = np.empty(NB, np.int64)  # bin -> global tile slot index
    core_of_rank = []
    for r in range(NB):
        blockpos = r // CORES
        cyc = r % CORES
        c = cyc if (blockpos % 2 == 0) else CORES - 1 - cyc
        core_of_rank.append(c)
    tile_ctr = np.zeros(CORES, np.int64)
    for r, b in enumerate(border):
        c = core_of_rank[r]
        assign[b] = c * TILES + tile_ctr[c]
        tile_ctr[c] += 1

    gpos = np.full(N, -1, np.int64)   # node -> global padded row
    for b in range(NB):
        gt = assign[b]
        c, t = gt // TILES, gt % TILES
        for p, nd in enumerate(bin_nodes[b]):
            gpos[nd] = c * SHP + t * 128 + p
    assert (gpos >= 0).all()

    owner = gpos // SHP
    local = gpos - owner * SHP

    # ---- per-core slot packing
    e_owner = owner[dst]
    K = 0
    per_core = []
    for c in range(CORES):
        sel = np.where(e_owner == c)[0]
        e_src_g = gpos[src[sel]]
        e_loc = local[dst[sel]]
        tid = e_loc >> 7
        cnt = np.bincount(tid, minlength=TILES)
        K = max(K, int(np.ceil(cnt.max() / 128)))
        per_core.append((e_src_g, e_loc, tid))

    CH = K + 1
    gsrcs, dstls, masks = [], [], []
    for c in range(CORES):
        e_src_g, e_loc, tid = per_core[c]
        gsrc_arr = np.zeros((128, TILES * K), np.int32)
        dstl_arr = np.full((128, TILES * CH), 200.0, np.float32)
        mask_arr = np.zeros((128, TILES, CH, 128), np.float32)
        order_e = np.argsort(tid, kind="stable")
        e_src_g, e_loc, tid = e_src_g[order_e], e_loc[order_e], tid[order_e]
        bounds = np.concatenate(
            [[0], np.cumsum(np.bincount(tid, minlength=TILES))])
        for t in range(TILES):
            seg = slice(bounds[t], bounds[t + 1])
            n_e = bounds[t + 1] - bounds[t]
            sl = np.arange(n_e)
            p = sl & 127
            k = sl >> 7
            rows = e_loc[seg] & 127
            gsrc_arr[p, t * K + k] = e_src_g[seg]
            dstl_arr[p, t * CH + k] = rows
            mask_arr[rows, t, k, p] = 1.0
            # self chunk: identity
            dstl_arr[:, t * CH + K] = np.arange(128)
            mask_arr[:, t, K, :] = np.eye(128, dtype=np.float32)
        gsrcs.append(gsrc_arr)
        dstls.append(dstl_arr)
        masks.append(
            mask_arr.reshape(128, TILES * CH * 128).astype(ml_dtypes.bfloat16))

    # ---- weights: per-layer sign-split permutation + a-folding
    Wls = [Wl0] + [Wl[i] for i in range(T - 1)]
    Wrs = [Wr0] + [Wr[i] for i in range(T - 1)]
    bls = [bl0] + [bl[i] for i in range(T - 1)]
    brs = [br0] + [br[i] for i in range(T - 1)]

    perms, ms_list = [], []
    for l in range(T):
        a = att[l]
        pos = np.where(a > 0)[0]
        neg = np.where(a <= 0)[0]
        perms.append(np.concatenate([pos, neg]))
        ms_list.append(len(pos))

    w_all = np.zeros((T, 128, 2 * D + 1), np.float32)
    corr = np.zeros((T, D), np.float32)
    bout = np.zeros((D, T), np.float32)
    for l in range(T):
        a = att[l]
        pi = perms[l]
        ap = a[pi]
        ap_safe = np.where(np.abs(ap) < 1e-12, 1e-12, ap)
        wl_r = Wls[l] if l == 0 else Wls[l][perms[l - 1], :]
        wr_r = Wrs[l] if l == 0 else Wrs[l][perms[l - 1], :]
        rows = wl_r.shape[0]
        w_all[l, :rows, 0:D] = wl_r[:, pi] * ap[None, :]
        w_all[l, :rows, D] = wl_r @ a
        w_all[l, :rows, D + 1 : 2 * D + 1] = wr_r[:, pi] * ap[None, :]
        corr[l] = 1.0 / ap_safe
        bout[:, l] = (bias[l] + bls[l])[pi]

    # ---- per-core inputs
    xT_full = np.zeros((CORES, DIN, SHP), np.float32)
    for c in range(CORES):
        selc = np.where(owner == c)[0]
        xT_full[c][:, local[selc]] = x[selc].T

    common = dict(
        w_all=w_all.astype(ml_dtypes.bfloat16),
        corr_d=corr, bout_d=bout)
    in_maps = []
    for c in range(CORES):
        in_maps.append(dict(
            common,
            xT_own=xT_full[c].astype(ml_dtypes.bfloat16),
            gsrc_d=gsrcs[c], dstl_d=dstls[c], mask_d=masks[c]))

    meta = dict(gpos=gpos, perm_last=perms[T - 1])
    return K, tuple(ms_list), in_maps, meta


_CACHE = {}


def kernel(**inputs) -> np.ndarray:
    out, _ = _run(inputs)
    return out


def _run(inputs, repeat=1, **kw):
    K, ms, in_maps, meta = _prep(inputs)
    key = (K, ms, repeat)
    if key not in _CACHE:
        _CACHE[key] = _build_nc(K, ms, repeat)
    nc = _CACHE[key]
    res = run_bass_kernel_spmd(nc, in_maps, core_ids=list(range(CORES)), **kw)
    cat = np.concatenate([res.results[c]["out"] for c in range(CORES)], axis=0)
    gpos = meta["gpos"]
    out = np.empty((N, D), np.float32)
    out[:, meta["perm_last"]] = cat[gpos]
    return out, res


# revision 6
# speedup vs baseline: 3.4607x; 3.4607x over previous
"""GATv2 x5 on 8 TRN2 cores — v2.

Per-core: nodes LPT-balanced into 49 tiles of 128 (K edge chunks per tile,
self-loops as a free 'identity' chunk). Per layer: prologue computes
xl_a = h @ (Wl diag(a) perm) and xr_a rows (bf16), AllGathers xl rows
[129 cols: xl_a|Ptilde]; edge stage per tile: indirect-gathers xl rows per
chunk, computes scores via PE adds (XL + mask@xr) into PSUM, Act-copies to
bf16, sigma-split abs-reduces give e = 0.6p + 0.4*sum(a*|u|) (LeakyReLU
algebra: LReLU(u) = 0.6u + 0.4|u|; exp(0.6 q_dst) cancels in softmax),
then one-hot-scaled scatter matmuls accumulate numerator+denominator.
"""
import sys
import numpy as np

sys.path.insert(0, "/opt/trn_rl_repo")

import ml_dtypes
import concourse.bass as bass
import concourse.bacc as bacc
import concourse.mybir as mybir
import concourse.tile as tile
from concourse.bass_utils import run_bass_kernel_spmd

F32 = mybir.dt.float32
BF16 = mybir.dt.bfloat16
I32 = mybir.dt.int32
AF = mybir.ActivationFunctionType
OP = mybir.AluOpType
AX = mybir.AxisListType

N = 50000
DIN = 7
D = 128
T = 5
CORES = 8
TILES = 49
SHP = TILES * 128          # 6272 padded nodes per core
NPG = CORES * SHP          # 50176 global padded rows
NEG = 0.2                  # LReLU(u) = 0.6u + 0.4|u| for slope 0.2
GRP = 4                    # score chunks per PSUM bank group


def _build_nc(K: int, ms: tuple, repeat: int = 1, ablate: tuple = ()):
    CH = K + 1  # + self chunk
    nc = bacc.Bacc("TRN2", target_bir_lowering=False, debug=False,
                   num_devices=CORES, num_swdge_queues=4)

    xT_own = nc.dram_tensor("xT_own", [DIN, SHP], BF16, kind="ExternalInput")
    w_all = nc.dram_tensor("w_all", [T, 128, 2 * D + 1], BF16,
                           kind="ExternalInput")
    corr_d = nc.dram_tensor("corr_d", [T, D], F32, kind="ExternalInput")
    bout_d = nc.dram_tensor("bout_d", [D, T], F32, kind="ExternalInput")
    gsrc_d = nc.dram_tensor("gsrc_d", [128, TILES * K], I32,
                            kind="ExternalInput")
    dstl_d = nc.dram_tensor("dstl_d", [128, TILES * CH], F32,
                            kind="ExternalInput")
    mask_d = nc.dram_tensor("mask_d", [128, TILES * CH * 128], BF16,
                            kind="ExternalInput")

    out_t = nc.dram_tensor("out", [SHP, D], F32, kind="ExternalOutput")

    with tile.TileContext(nc) as tc:
        with (
            tc.tile_pool(name="pers", bufs=1) as pers,
            tc.tile_pool(name="wl", bufs=2) as wl,
            tc.tile_pool(name="xe", bufs=3) as xe,       # XL / U / mask tiles
            tc.tile_pool(name="sb", bufs=4) as sbp,      # small tiles
            tc.tile_pool(name="pro", bufs=2, space="PSUM") as ppro,
            tc.tile_pool(name="pss", bufs=2, space="PSUM") as psc,
            tc.tile_pool(name="psa", bufs=3, space="PSUM") as psa,
            tc.tile_pool(name="pst", bufs=1, space="PSUM") as pstp,
            tc.tile_pool(name="dram", bufs=2, space="DRAM") as dp,
        ):
            # ---- persistent setup ----
            iota_c = pers.tile([128, 1], I32)
            nc.gpsimd.iota(iota_c[:], pattern=[[0, 1]], base=0,
                           channel_multiplier=1)
            iota_cf = pers.tile([128, 1], F32)
            nc.vector.tensor_copy(iota_cf[:], iota_c[:])
            iota_r = pers.tile([128, 128], I32)
            nc.gpsimd.iota(iota_r[:], pattern=[[1, 128]], base=0,
                           channel_multiplier=0)
            iota_rb = pers.tile([128, 128], BF16)
            nc.vector.tensor_copy(iota_rb[:], iota_r[:])
            ident_b = pers.tile([128, 128], BF16)
            nc.vector.tensor_scalar(
                out=ident_b[:], in0=iota_rb[:], scalar1=iota_cf[:],
                scalar2=None, op0=OP.is_equal)

            gsrc_sb = pers.tile([128, TILES * K], I32)
            nc.sync.dma_start(out=gsrc_sb[:], in_=gsrc_d[:, :])
            dstl_sb = pers.tile([128, TILES * CH], F32)
            nc.sync.dma_start(out=dstl_sb[:], in_=dstl_d[:, :])
            xT_sb = pers.tile([DIN, SHP], BF16)
            nc.sync.dma_start(out=xT_sb[:], in_=xT_own[:, :])

            hT = pers.tile([128, SHP], BF16)
            xl_store = pers.tile([128, TILES, D + 1], BF16)
            xr_store = pers.tile([128, TILES, D], BF16)

            def load_consts(l):
                w_sb = wl.tile([128, 2 * D + 1], BF16, tag="w")
                nc.sync.dma_start(out=w_sb[:], in_=w_all[l, :, :])
                corr_b = wl.tile([128, 128], F32, tag="corr")
                nc.sync.dma_start(
                    out=corr_b[:],
                    in_=corr_d[l : l + 1, :].partition_broadcast(128))
                bo_col = wl.tile([128, 1], F32, tag="boc")
                nc.sync.dma_start(out=bo_col[:], in_=bout_d[:, l : l + 1])
                bo_b = None
                if l == T - 1:
                    bo_b = wl.tile([128, 128], F32, tag="bob")
                    nc.sync.dma_start(
                        out=bo_b[:],
                        in_=bout_d[:, l : l + 1].transpose([1, 0])
                        .partition_broadcast(128))
                return w_sb, corr_b, bo_col, bo_b

            def prologue_tile(l, t, w_sb, xl_cc):
                ps_p = ppro.tile([128, 2 * D + 1], F32, space="PSUM",
                                 tag="pro")
                if l == 0:
                    nc.tensor.matmul(
                        out=ps_p[:], lhsT=xT_sb[:, t * 128 : (t + 1) * 128],
                        rhs=w_sb[:DIN, :], start=True, stop=True)
                else:
                    nc.tensor.matmul(
                        out=ps_p[:], lhsT=hT[:, t * 128 : (t + 1) * 128],
                        rhs=w_sb[:], start=True, stop=True)
                nc.vector.tensor_copy(xl_store[:, t, 0:D], ps_p[:, 0:D])
                nc.scalar.activation(
                    out=xl_store[:, t, D : D + 1],
                    in_=ps_p[:, D : D + 1], func=AF.Exp, scale=0.6)
                nc.vector.tensor_copy(
                    xr_store[:, t, :], ps_p[:, D + 1 : 2 * D + 1])
                nc.sync.dma_start(
                    out=xl_cc[t * 128 : (t + 1) * 128, :],
                    in_=xl_store[:, t, :])

            def all_gather(xl_cc):
                xl_full = dp.tile([NPG, D + 1], BF16, tag="xlfull",
                                  addr_space="Shared")
                if "ag" not in ablate:
                    nc.gpsimd.collective_compute(
                        "AllGather",
                        OP.bypass,
                        replica_groups=[list(range(CORES))],
                        ins=[xl_cc[:, :].opt()],
                        outs=[xl_full[:, :].opt()],
                    )
                return xl_full

            def edge_tile(l, t, m, xl_full, corr_b, bo_col, bo_b):
                XL = xe.tile([128, CH, D + 2], BF16, tag="XL")
                nc.vector.memset(XL[:, :, 0:1], 1.0)
                msk = xe.tile([128, CH * 128], BF16, tag="msk")
                nc.sync.dma_start(
                    out=msk[:],
                    in_=mask_d[:, t * CH * 128 : (t + 1) * CH * 128])
                for k in range(K if "gather" not in ablate else 0):
                    gi_inst = nc.gpsimd.indirect_dma_start(
                        out=XL[:, k, 1 : D + 2], out_offset=None,
                        in_=xl_full[:, :],
                        in_offset=bass.IndirectOffsetOnAxis(
                            ap=gsrc_sb[:, t * K + k : t * K + k + 1],
                            axis=0),
                    )
                    qn = k % 4
                    if qn:
                        try:
                            gi_inst.inst.queue = f"qPoolDynamic{qn}"
                        except AttributeError:
                            gi_inst.queue = f"qPoolDynamic{qn}"
                nc.vector.tensor_copy(XL[:, K, 1 : D + 2], xl_store[:, t, :])

                xr_t = xr_store[:, t, :]
                ps_a = psa.tile([128, D + 1], F32, space="PSUM", tag="a")
                ngrp = (CH + GRP - 1) // GRP
                for gi in range(ngrp):
                    g0 = gi * GRP
                    gw = min(GRP, CH - g0)
                    ps_s = psc.tile([128, GRP, D], F32, space="PSUM", tag="s")
                    for j in range(gw):
                        k = g0 + j
                        nc.tensor.matmul(
                            out=ps_s[:, j, :],
                            lhsT=msk[:, k * 128 : (k + 1) * 128],
                            rhs=xr_t, start=True, stop=False)
                        nc.tensor.matmul(
                            out=ps_s[:, j, :],
                            lhsT=ident_b[:],
                            rhs=XL[:, k, 1 : D + 1], start=False, stop=True)
                    U = sbp.tile([128, GRP, D], BF16, tag="U")
                    nc.scalar.activation(
                        out=U[:, :gw, :], in_=ps_s[:, :gw, :],
                        func=AF.Identity)
                    r1 = sbp.tile([128, GRP], F32, tag="r1")
                    r2n = sbp.tile([128, GRP], F32, tag="r2n")
                    if m > 0:
                        nc.vector.tensor_reduce(
                            out=r1[:, :gw], in_=U[:, :gw, 0:m], axis=AX.X,
                            op=OP.add, apply_absolute_value=True)
                    else:
                        nc.vector.memset(r1[:], 0.0)
                    if m < D:
                        nc.vector.tensor_reduce(
                            out=r2n[:, :gw], in_=U[:, :gw, m:D], axis=AX.X,
                            op=OP.add, apply_absolute_value=True, negate=True)
                    else:
                        nc.vector.memset(r2n[:], 0.0)
                    e4 = sbp.tile([128, GRP], F32, tag="e4")
                    nc.vector.tensor_tensor(
                        out=e4[:, :gw], in0=r1[:, :gw], in1=r2n[:, :gw],
                        op=OP.add)
                    exg = sbp.tile([128, GRP], F32, tag="ex")
                    nc.scalar.activation(
                        out=exg[:, :gw], in_=e4[:, :gw], func=AF.Exp,
                        scale=0.4)
                    nc.vector.tensor_tensor(
                        out=exg[:, :gw], in0=exg[:, :gw],
                        in1=XL[:, g0 : g0 + gw, D + 1], op=OP.mult)
                    for j in range(gw):
                        k = g0 + j
                        Oc = sbp.tile([128, 128], BF16, tag="Oc")
                        nc.vector.tensor_scalar(
                            out=Oc[:], in0=iota_rb[:],
                            scalar1=dstl_sb[:, t * CH + k : t * CH + k + 1],
                            scalar2=exg[:, j : j + 1],
                            op0=OP.is_equal, op1=OP.mult)
                        nc.tensor.matmul(
                            out=ps_a[:], lhsT=Oc[:], rhs=XL[:, k, 0 : D + 1],
                            start=(k == 0), stop=(k == CH - 1))

                rec = sbp.tile([128, 1], F32, tag="rec")
                nc.vector.reciprocal(rec[:], ps_a[:, 0:1])
                if l < T - 1:
                    h1 = sbp.tile([128, D], BF16, tag="h1")
                    nc.vector.scalar_tensor_tensor(
                        out=h1[:], in0=ps_a[:, 1 : D + 1], scalar=rec[:],
                        in1=corr_b[:], op0=OP.mult, op1=OP.mult)
                    ps_t = pstp.tile([128, 128], BF16, space="PSUM", tag="tr")
                    nc.tensor.transpose(out=ps_t[:], in_=h1[:],
                                        identity=ident_b[:])
                    nc.scalar.activation(
                        out=hT[:, t * 128 : (t + 1) * 128], in_=ps_t[:],
                        func=AF.Relu, bias=bo_col[:], scale=1.0)
                else:
                    h1f = sbp.tile([128, D], F32, tag="h1f")
                    nc.vector.scalar_tensor_tensor(
                        out=h1f[:], in0=ps_a[:, 1 : D + 1], scalar=rec[:],
                        in1=corr_b[:], op0=OP.mult, op1=OP.mult)
                    o_sb = sbp.tile([128, D], F32, tag="o")
                    nc.vector.tensor_tensor(
                        out=o_sb[:], in0=h1f[:], in1=bo_b[:], op=OP.add)
                    nc.sync.dma_start(
                        out=out_t[t * 128 : (t + 1) * 128, :], in_=o_sb[:])

            for rep in range(repeat):
                # layer 0 prologue
                consts = load_consts(0)
                xl_cc = dp.tile([SHP, D + 1], BF16, tag="xlcc")
                for t in range(TILES):
                    prologue_tile(0, t, consts[0], xl_cc)
                xl_full = all_gather(xl_cc)

                for l in range(T):
                    w_sb, corr_b, bo_col, bo_b = consts
                    if l < T - 1:
                        consts_next = load_consts(l + 1)
                        xl_cc_next = dp.tile([SHP, D + 1], BF16, tag="xlcc")
                    # fused: edge(l, t) then prologue(l+1, t)
                    for t in range(TILES):
                        edge_tile(l, t, ms[l], xl_full, corr_b, bo_col, bo_b)
                        if l < T - 1:
                            prologue_tile(l + 1, t, consts_next[0], xl_cc_next)
                    if l < T - 1:
                        xl_full = all_gather(xl_cc_next)
                        consts = consts_next

    nc.compile()
    return nc


def _prep(inputs):
    x = np.asarray(inputs["x"], np.float32)
    ei = np.asarray(inputs["edge_index"]).astype(np.int64)
    Wl0 = np.asarray(inputs["Wl0"], np.float32)
    Wr0 = np.asarray(inputs["Wr0"], np.float32)
    Wl = np.asarray(inputs["Wl"], np.float32)
    Wr = np.asarray(inputs["Wr"], np.float32)
    bl0 = np.asarray(inputs["bl0"], np.float32)
    br0 = np.asarray(inputs["br0"], np.float32)
    bl = np.asarray(inputs["bl"], np.float32)
    br = np.asarray(inputs["br"], np.float32)
    att = np.asarray(inputs["att"], np.float32)
    bias = np.asarray(inputs["bias"], np.float32)

    src, dst = ei[0], ei[1]
    E = src.shape[0]

    # ---- LPT node placement: balance in-degree over 392 (core,tile) bins
    deg = np.bincount(dst, minlength=N).astype(np.int64)
    NB = CORES * TILES
    order = np.argsort(-deg, kind="stable")
    bin_load = np.zeros(NB, np.int64)
    bin_cnt = np.zeros(NB, np.int32)
    bin_nodes = [[] for _ in range(NB)]
    import heapq
    heap = [(0, 0, b) for b in range(NB)]
    heapq.heapify(heap)
    for nd in order:
        while True:
            load, cnt, b = heapq.heappop(heap)
            if (bin_cnt[b] < 128 and load == bin_load[b]
                    and cnt == bin_cnt[b]):
                break
        bin_nodes[b].append(nd)
        bin_cnt[b] += 1
        bin_load[b] += deg[nd]
        if bin_cnt[b] < 128:
            heapq.heappush(heap, (bin_load[b], int(bin_cnt[b]), b))

    # bins -> (core, tile): snake by load for core balance
    border = np.argsort(-bin_load, kind="stable")
    assign = np.empty(NB, np.int64)  # bin -> global tile slot index
    core_of_rank = []
    for r in range(NB):
        blockpos = r // CORES
        cyc = r % CORES
        c = cyc if (blockpos % 2 == 0) else CORES - 1 - cyc
        core_of_rank.append(c)
    tile_ctr = np.zeros(CORES, np.int64)
    for r, b in enumerate(border):
        c = core_of_rank[r]
        assign[b] = c * TILES + tile_ctr[c]
        tile_ctr[c] += 1

    gpos = np.full(N, -1, np.int64)   # node -> global padded row
    for b in range(NB):
        gt = assign[b]
        c, t = gt // TILES, gt % TILES
        for p, nd in enumerate(bin_nodes[b]):
            gpos[nd] = c * SHP + t * 128 + p
    assert (gpos >= 0).all()

    owner = gpos // SHP
    local = gpos - owner * SHP

    # ---- per-core slot packing
    e_owner = owner[dst]
    K = 0
    per_core = []
    for c in range(CORES):
        sel = np.where(e_owner == c)[0]
        e_src_g = gpos[src[sel]]
        e_loc = local[dst[sel]]
        tid = e_loc >> 7
        cnt = np.bincount(tid, minlength=TILES)
        K = max(K, int(np.ceil(cnt.max() / 128)))
        per_core.append((e_src_g, e_loc, tid))

    CH = K + 1
    gsrcs, dstls, masks = [], [], []
    for c in range(CORES):
        e_src_g, e_loc, tid = per_core[c]
        gsrc_arr = np.zeros((128, TILES * K), np.int32)
        dstl_arr = np.full((128, TILES * CH), 200.0, np.float32)
        mask_arr = np.zeros((128, TILES, CH, 128), np.float32)
        order_e = np.argsort(tid, kind="stable")
        e_src_g, e_loc, tid = e_src_g[order_e], e_loc[order_e], tid[order_e]
        bounds = np.concatenate(
            [[0], np.cumsum(np.bincount(tid, minlength=TILES))])
        for t in range(TILES):
            seg = slice(bounds[t], bounds[t + 1])
            n_e = bounds[t + 1] - bounds[t]
            sl = np.arange(n_e)
            p = sl & 127
            k = sl >> 7
            rows = e_loc[seg] & 127
            gsrc_arr[p, t * K + k] = e_src_g[seg]
            dstl_arr[p, t * CH + k] = rows
            mask_arr[rows, t, k, p] = 1.0
            # self chunk: identity
            dstl_arr[:, t * CH + K] = np.arange(128)
            mask_arr[:, t, K, :] = np.eye(128, dtype=np.float32)
        gsrcs.append(gsrc_arr)
        dstls.append(dstl_arr)
        masks.append(
            mask_arr.reshape(128, TILES * CH * 128).astype(ml_dtypes.bfloat16))

    # ---- weights: per-layer sign-split permutation + a-folding
    Wls = [Wl0] + [Wl[i] for i in range(T - 1)]
    Wrs = [Wr0] + [Wr[i] for i in range(T - 1)]
    bls = [bl0] + [bl[i] for i in range(T - 1)]
    brs = [br0] + [br[i] for i in range(T - 1)]

    perms, ms_list = [], []
    for l in range(T):
        a = att[l]
        pos = np.where(a > 0)[0]
        neg = np.where(a <= 0)[0]
        perms.append(np.concatenate([pos, neg]))
        ms_list.append(len(pos))

    w_all = np.zeros((T, 128, 2 * D + 1), np.float32)
    corr = np.zeros((T, D), np.float32)
    bout = np.zeros((D, T), np.float32)
    for l in range(T):
        a = att[l]
        pi = perms[l]
        ap = a[pi]
        ap_safe = np.where(np.abs(ap) < 1e-12, 1e-12, ap)
        wl_r = Wls[l] if l == 0 else Wls[l][perms[l - 1], :]
        wr_r = Wrs[l] if l == 0 else Wrs[l][perms[l - 1], :]
        rows = wl_r.shape[0]
        w_all[l, :rows, 0:D] = wl_r[:, pi] * ap[None, :]
        w_all[l, :rows, D] = wl_r @ a
        w_all[l, :rows, D + 1 : 2 * D + 1] = wr_r[:, pi] * ap[None, :]
        corr[l] = 1.0 / ap_safe
        bout[:, l] = (bias[l] + bls[l])[pi]

    # ---- per-core inputs
    xT_full = np.zeros((CORES, DIN, SHP), np.float32)
    for c in range(CORES):
        selc = np.where(owner == c)[0]
        xT_full[c][:, local[selc]] = x[selc].T

    common = dict(
        w_all=w_all.astype(ml_dtypes.bfloat16),
        corr_d=corr, bout_d=bout)
    in_maps = []
    for c in range(CORES):
        in_maps.append(dict(
            common,
            xT_own=xT_full[c].astype(ml_dtypes.bfloat16),
            gsrc_d=gsrcs[c], dstl_d=dstls[c], mask_d=masks[c]))

    meta = dict(gpos=gpos, perm_last=perms[T - 1])
    return K, tuple(ms_list), in_maps, meta


_CACHE = {}


def kernel(**inputs) -> np.ndarray:
    out, _ = _run(inputs)
    return out


def _run(inputs, repeat=1, **kw):
    K, ms, in_maps, meta = _prep(inputs)
    key = (K, ms, repeat)
    if key not in _CACHE:
        _CACHE[key] = _build_nc(K, ms, repeat)
    nc = _CACHE[key]
    res = run_bass_kernel_spmd(nc, in_maps, core_ids=list(range(CORES)), **kw)
    cat = np.concatenate([res.results[c]["out"] for c in range(CORES)], axis=0)
    gpos = meta["gpos"]
    out = np.empty((N, D), np.float32)
    out[:, meta["perm_last"]] = cat[gpos]
    return out, res
